# revision 72
# baseline (speedup 1.0000x reference)
"""PhotonicCore micro-ring transmission kernel for 8x TRN2 NeuronCores.

Math:
    phi = phi0 + X[b,j] + P[i,j]
    trans = (t^2 - 2at*cos(phi) + a^2) / (1 - 2at*cos(phi) + (at)^2)
          = 1 - gamma / d,   d = D - Bc*cos(phi),  D = 1+(at)^2, Bc = 2at,
            gamma = kappa*(1-a^2)
    out[b,i] = sum_j trans = 256 - gamma * sum_j 1/d[b,i,j]

Device strategy (data-parallel over batch, 256 rows/core):
    cos(phi) = cos(u)cos(p) - sin(u)sin(p)  with u = phi0 + X  ==>  for a
    fixed j, -Bc*cos(phi)[b,i] is a rank-2 outer product.  TensorE matmuls
    (K=12, fp16 hi/lo split => ~fp32-exact products) build -Bc*cos(phi)
    for 8 j-columns into a 4-bank PSUM group [128b x 2048].  ScalarE does
    the one mandatory per-element pass r = Reciprocal(psum + D) (measured
    2.7e-5 max rel err on our d range) into SBUF; the j-reduction runs as
    two independent accumulation chains on VectorE and GpSimdE (S += r),
    merged + folded + scaled by VectorE per 128-row block.
"""

import numpy as np

import concourse.bass as bass
import concourse.mybir as mybir
from concourse.bass_utils import run_bass_kernel_spmd

# ---- problem constants (hardcoded; kernel.py must be self-contained) ----
B, I_DIM, O_DIM = 2048, 256, 256
N_CORES = 8
BL = B // N_CORES  # 256 batch rows per core

KAPPA = 0.1
LOSS_A = 0.99
N_EFF = 3.48
LAMBDA = 1.55e-6
RADIUS = 5e-6

_t = np.sqrt(1.0 - KAPPA)
_a = LOSS_A
_x = _a * _t
BC = 2.0 * _a * _t
D_CONST = 1.0 + _x * _x
GAMMA = KAPPA * (1.0 - _a * _a)
PHI0 = float(np.float32((2.0 * np.pi * N_EFF * (2.0 * np.pi * RADIUS) / LAMBDA) % (2.0 * np.pi)))

N_PAIRS = 128   # j pairs per core (256 j / 2)
KK = 12         # contraction rows per matmul (6 per j)
PPG = 4         # pairs per PSUM group (4 pairs = 8 j = [128, 2048] = 4 banks)
GQ = N_PAIRS // PPG          # 32 groups per bt
NRT = 6         # r-tile rotation depth
NDG = 2         # PSUM d-group rotation depth


def _split16(v):
    hi = v.astype(np.float16)
    lo = (v - hi.astype(np.float64)).astype(np.float16)
    return hi, lo


def _pack2(per_pair, w):
    """Split per-pair [12, w] blocks into even/odd-pair compact tensors
    [12, 64*w] (DRAM stays compact; the DMA scatters even pairs to SBUF
    partition base 0 and odd pairs to base 64, where matmul can slice
    [12, w] blocks at legal partition bases)."""
    arr = per_pair.reshape(KK, N_PAIRS, w)
    even = np.ascontiguousarray(arr[:, 0::2, :].reshape(KK, 64 * w))
    odd = np.ascontiguousarray(arr[:, 1::2, :].reshape(KK, 64 * w))
    return even, odd


def _host_wl(x_shard):
    """lhsT source: even/odd compact [12, 64*256] fp16 pair (see _pack2)."""
    u = PHI0 + x_shard.astype(np.float64)          # [256b, 256j]
    cu = -BC * np.cos(u)
    su = BC * np.sin(u)
    cu_hi, cu_lo = _split16(cu)
    su_hi, su_lo = _split16(su)
    rows = np.stack([cu_hi, cu_hi, cu_lo, su_hi, su_hi, su_lo], 0)  # [6,b,j]
    t_even = rows[:, :, 0::2].transpose(0, 2, 1)   # [6, 128p, 256b]
    t_odd = rows[:, :, 1::2].transpose(0, 2, 1)
    T = np.stack([t_even, t_odd], 0)               # [2, 6, 128p, 256b]
    # [h(2), s(6), p, b] -> [6h+s, p*256+b]
    return _pack2(T.reshape(KK, N_PAIRS * 256), 256)


def _host_rr(phase_offset):
    """rhs source: even/odd compact [12, 64*512] fp16 pair, block-diagonal
    per j-pair (see _pack2)."""
    p = phase_offset.astype(np.float64)            # [256i, 256j]
    cp_hi, cp_lo = _split16(np.cos(p))
    sp_hi, sp_lo = _split16(np.sin(p))
    rows = np.stack([cp_hi, cp_lo, cp_hi, sp_hi, sp_lo, sp_hi], 0)  # [6, i, j]
    r_even = rows[:, :, 0::2].transpose(0, 2, 1)   # [6, 128p, 256i]
    r_odd = rows[:, :, 1::2].transpose(0, 2, 1)
    Z = np.zeros((2, 6, N_PAIRS, 2, 256), np.float16)      # [h, s, p, h', i]
    Z[0, :, :, 0, :] = r_even
    Z[1, :, :, 1, :] = r_odd
    return _pack2(Z.reshape(KK, N_PAIRS * 512), 512)


def _act_recip(nc, out, in_, bias):
    """out = 1/(in_ + bias) on ScalarE via direct InstActivation emission."""
    eng = nc.scalar
    inputs = [eng.lower_ap(in_)]
    for arg in [bias, 1.0, 0.0]:  # bias, scale, alpha
        inputs.append(mybir.ImmediateValue(dtype=mybir.dt.float32, value=float(arg)))
    return eng.add_instruction(
        mybir.InstActivation(
            name=nc.get_next_instruction_name(),
            func=mybir.ActivationFunctionType.Reciprocal,
            ins=inputs,
            outs=[eng.lower_ap(out)],
        )
    )


# which engine accumulates group q (within a bt): True -> GpSimd, else Vector.
# GP is ~2x slower per op than DVE: give it every 3rd group with slack, none
# near the bt end so it can pre-fold its accumulator off the critical tail.
def _is_gp(q):
    return q in (1, 4, 6, 9, 11, 14, 16, 19, 21, 24, 26)


N_INJ = 0                                         # trailing groups TensorE injects
_IS_INJ = lambda q: q >= GQ - N_INJ


def _is_dv(q):
    return not _is_gp(q) and not _IS_INJ(q)


_N_GP = sum(1 for q in range(GQ) if _is_gp(q))    # GP groups per bt (11)
_N_DV = sum(1 for q in range(GQ) if _is_dv(q))    # DVE groups per bt (19)
_GP_OPS_BT = _N_GP + 2                            # adds + 2 pre-folds


def build_nc(n_reps=1):
    """Build the per-core Bass graph. n_reps>1 repeats compute for timing."""
    f16, f32 = mybir.dt.float16, mybir.dt.float32
    add, mult = mybir.AluOpType.add, mybir.AluOpType.mult

    f32r = mybir.dt.float32r
    nc = bass.Bass("TRN2", target_bir_lowering=False, debug=False)
    wle = nc.dram_tensor("WLE", [KK, 64 * 256], f16, kind="ExternalInput").ap()
    wlo = nc.dram_tensor("WLO", [KK, 64 * 256], f16, kind="ExternalInput").ap()
    rre = nc.dram_tensor("RRE", [KK, 64 * 512], f16, kind="ExternalInput").ap()
    rro = nc.dram_tensor("RRO", [KK, 64 * 512], f16, kind="ExternalInput").ap()
    idm = nc.dram_tensor("IDM", [128, 128], f32, kind="ExternalInput").ap()
    y = nc.dram_tensor("OUT", [BL, 256], f32, kind="ExternalOutput").ap()

    # input DMA chunk plan over 64 col-blocks: tiny first chunks so the first
    # matmuls (and hence ScalarE) start as early as possible.
    # chunk 0 (first 2 col-blocks) goes out on three queues in parallel
    # (sync x2 + scalar + gpsimd, tracked on dma0_sem); the remaining chunks
    # stream in-order on sync's queue with growing sizes.
    CHUNK_PLAN = [2, 4, 8, 16, 34]
    assert sum(CHUNK_PLAN) == 64
    IN_DMAS_SYNC = 2 + 4 * (len(CHUNK_PLAN) - 1) + 1  # dma_sem incs from sync (+IDM)
    # cb -> dma_sem value once its (sync) chunk fully landed
    _cb_full_v = {}
    acc = CHUNK_PLAN[0]
    v = 32                      # sync's two chunk-0 DMAs
    for n_cb in CHUNK_PLAN[1:]:
        v += 64
        for cb in range(acc, acc + n_cb):
            _cb_full_v[cb] = v
        acc += n_cb

    def _wl_slice(p, bt):
        base = 64 * (p % 2)
        col = (p // 2) * 256 + bt * 128
        return (base, col)

    FD = PPG * 512   # 2048 free elems per group

    from contextlib import ExitStack
    with ExitStack() as st:
        ec = st.enter_context
        wls = ec(nc.sbuf_tensor("wls", [128, 64 * 256], f16))
        rrs = ec(nc.sbuf_tensor("rrs", [128, 64 * 512], f16))
        ids = ec(nc.sbuf_tensor("ids", [128, 128], f32))
        rt = [ec(nc.sbuf_tensor(f"rt{i}", [128, FD], f32)) for i in range(NRT)]
        s_dv = ec(nc.sbuf_tensor("s_dv", [128, FD], f32))
        s_gp = [ec(nc.sbuf_tensor(f"s_gp{i}", [128, FD], f32)) for i in range(2)]
        o_t = [ec(nc.sbuf_tensor(f"o{i}", [128, 256], f32)) for i in range(2)]
        dg = [ec(nc.psum_tensor(f"dg{i}", [128, FD], f32)) for i in range(NDG)]
        dma_sem = ec(nc.semaphore())
        dma0_sem = ec(nc.semaphore())  # chunk-0 DMAs on scalar/gpsimd queues
        mm_sem = ec(nc.semaphore())    # d-matmuls
        act_sem = ec(nc.semaphore())   # ACT recip groups
        dv_sem = ec(nc.semaphore())    # DVE chain adds
        gp_sem = ec(nc.semaphore())    # GP chain adds
        inj_sem = ec(nc.semaphore())   # TensorE tail injects
        sp_sem = ec(nc.semaphore())    # DVE consumed tail-inject PSUM
        o_sem = ec(nc.semaphore())     # per-bt finished outputs
        blk = ec(nc.Block())

        # ---- static schedules / sem count maps ----
        # global group index: gg = (rep*2 + bt)*GQ + q
        n_bt = 2 * n_reps

        def gg_of(rep, bt, q):
            return (rep * 2 + bt) * GQ + q

        # chain-consumer sem value after the accumulate op for global group g.
        # GP emits _GP_OPS_BT ops per bt (adds + 2 pre-folds); DVE emits _N_DV
        # chain adds per bt (tail fold/merge ops do not inc dv_sem); the last
        # N_INJ groups are injected by TensorE (4 inj_sem incs per group).
        cons = {}   # g -> (sem_name, value)
        for g in range(n_bt * GQ):
            blk_i, q = divmod(g, GQ)
            if _is_gp(q):
                cons[g] = ("gp", blk_i * _GP_OPS_BT + sum(
                    1 for qq in range(q + 1) if _is_gp(qq)))
            elif _IS_INJ(q):
                cons[g] = ("inj", blk_i * 4 * N_INJ
                           + 4 * (q - (GQ - N_INJ) + 1))
            else:
                cons[g] = ("dv", blk_i * _N_DV + sum(
                    1 for qq in range(q + 1) if _is_dv(qq)))

        # chunk 0's four DMAs issue from four different engines' queues so
        # they land in parallel (shortest path to the first matmul); sync
        # then waits for them before streaming the remaining chunks in-order
        # (keeps the cumulative dma_sem thresholds valid).
        def _chunk_slices(acc, n_cb):
            lo, hi = acc * 256, (acc + n_cb) * 256
            lo2, hi2 = acc * 512, (acc + n_cb) * 512
            return [(wls[0:KK, lo:hi], wle[:, lo:hi]),
                    (rrs[0:KK, lo2:hi2], rre[:, lo2:hi2]),
                    (wls[64:64 + KK, lo:hi], wlo[:, lo:hi]),
                    (rrs[64:64 + KK, lo2:hi2], rro[:, lo2:hi2])]

        @blk.sync
        def _(sync):
            c0 = _chunk_slices(0, CHUNK_PLAN[0])
            sync.dma_start(c0[0][0], c0[0][1]).then_inc(dma_sem, 16)
            sync.dma_start(c0[3][0], c0[3][1]).then_inc(dma_sem, 16)
            acc = CHUNK_PLAN[0]
            for n_cb in CHUNK_PLAN[1:]:
                for dst, src in _chunk_slices(acc, n_cb):
                    sync.dma_start(dst, src).then_inc(dma_sem, 16)
                acc += n_cb
            sync.dma_start(ids[:], idm[:]).then_inc(dma_sem, 16)
            for rep in range(n_reps):
                for bt in range(2):
                    sync.wait_ge(o_sem, rep * 2 + bt + 1)
                    sync.dma_start(y[bt * 128:(bt + 1) * 128, :], o_t[bt][:]
                                   ).then_inc(dma_sem, 16)

        @blk.tensor
        def _(tensor):
            for rep in range(n_reps):
                for bt in range(2):
                    blk_i = rep * 2 + bt
                    for q in range(GQ):
                        gg = gg_of(rep, bt, q)
                        if gg >= NDG:
                            # ACT must have consumed dg[gg % NDG]
                            tensor.wait_ge(act_sem, gg - NDG + 1)
                        if q == 0 and blk_i > 0 and N_INJ:
                            # tail-inject region (dg[even][:, 0:512]) of the
                            # previous bt must have been read by DVE
                            tensor.wait_ge(sp_sem, blk_i)
                        for h in range(PPG):
                            p = q * PPG + h
                            if rep == 0 and bt == 0:
                                cb = p // 2
                                if cb < CHUNK_PLAN[0]:
                                    tensor.wait_ge(dma_sem, 32)
                                    tensor.wait_ge(dma0_sem, 32)
                                else:
                                    tensor.wait_ge(dma_sem, _cb_full_v[cb])
                            base, col = _wl_slice(p, bt)
                            tensor.matmul(
                                dg[gg % NDG][:, h * 512:(h + 1) * 512],
                                wls[base:base + KK, col:col + 128],
                                rrs[base:base + KK, (p // 2) * 512:(p // 2) * 512 + 512],
                                start=True, stop=True,
                            ).then_inc(mm_sem, 1)
                    # tail injects: accumulate the last N_INJ groups' r-tiles
                    # into dg[even][:, 0:512] (one PSUM bank) via float32r
                    # identity matmuls -- folds 2048 -> 512 for free.
                    for k in range(N_INJ):
                        q = GQ - N_INJ + k
                        gg = gg_of(rep, bt, q)
                        s_ps = dg[gg_of(rep, bt, GQ - N_INJ) % NDG]
                        tensor.wait_ge(act_sem, gg + 1)
                        for c in range(4):
                            tensor.matmul(
                                s_ps[:, 0:512],
                                ids[:].bitcast(f32r),
                                rt[gg % NRT][:, c * 512:(c + 1) * 512].bitcast(f32r),
                                start=(k == 0 and c == 0),
                                stop=(k == N_INJ - 1 and c == 3),
                                skip_group_check=True,
                            ).then_inc(inj_sem, 1)

        @blk.scalar
        def _(scalar):
            dst, src = _chunk_slices(0, CHUNK_PLAN[0])[1]
            scalar.dma_start(dst, src).then_inc(dma0_sem, 16)
            sems = {"gp": gp_sem, "dv": dv_sem, "inj": inj_sem}
            for rep in range(n_reps):
                for bt in range(2):
                    for q in range(GQ):
                        gg = gg_of(rep, bt, q)
                        if gg >= NRT:
                            # accumulation chain must have consumed rt[gg % NRT]
                            sem_name, val = cons[gg - NRT]
                            scalar.wait_ge(sems[sem_name], val)
                        if gg == 0:
                            # split the very first group so ScalarE starts
                            # after only 2 of its 4 matmuls have landed
                            scalar.wait_ge(mm_sem, 2)
                            _act_recip(nc, rt[0][:, 0:1024], dg[0][:, 0:1024],
                                       D_CONST)
                            scalar.wait_ge(mm_sem, 4)
                            _act_recip(nc, rt[0][:, 1024:2048], dg[0][:, 1024:2048],
                                       D_CONST).then_inc(act_sem, 1)
                            continue
                        scalar.wait_ge(mm_sem, PPG * (gg + 1))
                        _act_recip(nc, rt[gg % NRT][:], dg[gg % NDG][:], D_CONST
                                   ).then_inc(act_sem, 1)

        @blk.gpsimd
        def _(gpsimd):
            dst, src = _chunk_slices(0, CHUNK_PLAN[0])[2]
            gpsimd.dma_start(dst, src).then_inc(dma0_sem, 16)
            for rep in range(n_reps):
                for bt in range(2):
                    sg = s_gp[bt]
                    if rep > 0:
                        # previous rep's merge for this bt parity consumed sg
                        gpsimd.wait_ge(o_sem, 2 * (rep - 1) + bt + 1)
                    first = True
                    for q in range(GQ):
                        if not _is_gp(q):
                            continue
                        gg = gg_of(rep, bt, q)
                        gpsimd.wait_ge(act_sem, gg + 1)
                        if first:
                            gpsimd.tensor_copy(sg[:], rt[gg % NRT][:]
                                               ).then_inc(gp_sem, 1)
                        else:
                            gpsimd.tensor_tensor(sg[:], sg[:], rt[gg % NRT][:],
                                                 add).then_inc(gp_sem, 1)
                        first = False
                    # pre-fold own accumulator 2048 -> 512 (off the tail path)
                    gpsimd.tensor_tensor(sg[:, 0:1024], sg[:, 0:1024],
                                         sg[:, 1024:2048], add).then_inc(gp_sem, 1)
                    gpsimd.tensor_tensor(sg[:, 0:512], sg[:, 0:512],
                                         sg[:, 512:1024], add).then_inc(gp_sem, 1)

        @blk.vector
        def _(vector):
            for rep in range(n_reps):
                for bt in range(2):
                    blk_i = rep * 2 + bt
                    first = True
                    for q in range(GQ):
                        if not _is_dv(q):
                            continue
                        gg = gg_of(rep, bt, q)
                        vector.wait_ge(act_sem, gg + 1)
                        if first:
                            vector.tensor_copy(s_dv[:], rt[gg % NRT][:]
                                               ).then_inc(dv_sem, 1)
                        else:
                            vector.tensor_tensor(s_dv[:], s_dv[:], rt[gg % NRT][:],
                                                 add).then_inc(dv_sem, 1)
                        first = False
                    # fold 2048 -> 512, merge GP, then the injected PSUM bank
                    vector.tensor_tensor(s_dv[:, 0:1024], s_dv[:, 0:1024],
                                         s_dv[:, 1024:2048], add)
                    vector.tensor_tensor(s_dv[:, 0:512], s_dv[:, 0:512],
                                         s_dv[:, 512:1024], add)
                    vector.wait_ge(gp_sem, (blk_i + 1) * _GP_OPS_BT)
                    vector.tensor_tensor(s_dv[:, 0:512], s_dv[:, 0:512],
                                         s_gp[bt][:, 0:512], add)
                    if N_INJ:
                        s_ps = dg[gg_of(rep, bt, GQ - N_INJ) % NDG]
                        vector.wait_ge(inj_sem, (blk_i + 1) * 4 * N_INJ)
                        vector.tensor_tensor(s_dv[:, 0:512], s_dv[:, 0:512],
                                             s_ps[:, 0:512], add).then_inc(sp_sem, 1)
                    if rep > 0:
                        vector.wait_ge(dma_sem,
                                       16 * (IN_DMAS_SYNC + 2 * (rep - 1) + bt + 1))
                    vector.tensor_tensor(o_t[bt][:], s_dv[:, 0:256], s_dv[:, 256:512],
                                         add)
                    vector.tensor_scalar(o_t[bt][:], o_t[bt][:],
                                         -GAMMA, float(I_DIM), mult, add
                                         ).then_inc(o_sem, 1)

    return nc


_NC_CACHE = {}


def _get_nc(n_reps=1):
    if n_reps not in _NC_CACHE:
        _NC_CACHE[n_reps] = build_nc(n_reps)
    return _NC_CACHE[n_reps]


def kernel(input_matrix, phase_offset, n_reps=1):
    input_matrix = np.asarray(input_matrix, np.float32)
    phase_offset = np.asarray(phase_offset, np.float32)
    rre, rro = _host_rr(phase_offset)
    idm = np.eye(128, dtype=np.float32)
    in_maps = []
    for c in range(N_CORES):
        wle, wlo = _host_wl(input_matrix[c * BL:(c + 1) * BL, :])
        in_maps.append({"WLE": wle, "WLO": wlo, "RRE": rre, "RRO": rro,
                        "IDM": idm})
    nc = _get_nc(n_reps)
    res = run_bass_kernel_spmd(nc, in_maps, core_ids=list(range(N_CORES)))
    out = np.empty((B, O_DIM), np.float32)
    for c in range(N_CORES):
        out[c * BL:(c + 1) * BL, :] = res.results[c]["OUT"]
    return out


# revision 75
# speedup vs baseline: 1.0044x; 1.0044x over previous
"""PhotonicCore micro-ring transmission kernel for 8x TRN2 NeuronCores.

Math:
    phi = phi0 + X[b,j] + P[i,j]
    trans = (t^2 - 2at*cos(phi) + a^2) / (1 - 2at*cos(phi) + (at)^2)
          = 1 - gamma / d,   d = D - Bc*cos(phi),  D = 1+(at)^2, Bc = 2at,
            gamma = kappa*(1-a^2)
    out[b,i] = sum_j trans = 256 - gamma * sum_j 1/d[b,i,j]

Device strategy (data-parallel over batch, 256 rows/core):
    cos(phi) = cos(u)cos(p) - sin(u)sin(p)  with u = phi0 + X  ==>  for a
    fixed j, -Bc*cos(phi)[b,i] is a rank-2 outer product.  TensorE matmuls
    (K=12, fp16 hi/lo split => ~fp32-exact products) build -Bc*cos(phi)
    for 8 j-columns into a 4-bank PSUM group [128b x 2048].  ScalarE does
    the one mandatory per-element pass r = Reciprocal(psum + D) (measured
    2.7e-5 max rel err on our d range) into SBUF; the j-reduction runs as
    two independent accumulation chains on VectorE and GpSimdE (S += r),
    merged + folded + scaled by VectorE per 128-row block.
"""

import numpy as np

import concourse.bass as bass
import concourse.mybir as mybir
from concourse.bass_utils import run_bass_kernel_spmd

# ---- problem constants (hardcoded; kernel.py must be self-contained) ----
B, I_DIM, O_DIM = 2048, 256, 256
N_CORES = 8
BL = B // N_CORES  # 256 batch rows per core

KAPPA = 0.1
LOSS_A = 0.99
N_EFF = 3.48
LAMBDA = 1.55e-6
RADIUS = 5e-6

_t = np.sqrt(1.0 - KAPPA)
_a = LOSS_A
_x = _a * _t
BC = 2.0 * _a * _t
D_CONST = 1.0 + _x * _x
GAMMA = KAPPA * (1.0 - _a * _a)
PHI0 = float(np.float32((2.0 * np.pi * N_EFF * (2.0 * np.pi * RADIUS) / LAMBDA) % (2.0 * np.pi)))

N_PAIRS = 128   # j pairs per core (256 j / 2)
KK = 12         # contraction rows per matmul (6 per j)
PPG = 4         # pairs per PSUM group (4 pairs = 8 j = [128, 2048] = 4 banks)
GQ = N_PAIRS // PPG          # 32 groups per bt
NRT = 6         # r-tile rotation depth
NDG = 2         # PSUM d-group rotation depth


def _split16(v):
    hi = v.astype(np.float16)
    lo = (v - hi.astype(np.float64)).astype(np.float16)
    return hi, lo


def _pack2(per_pair, w):
    """Split per-pair [12, w] blocks into even/odd-pair compact tensors
    [12, 64*w] (DRAM stays compact; the DMA scatters even pairs to SBUF
    partition base 0 and odd pairs to base 64, where matmul can slice
    [12, w] blocks at legal partition bases)."""
    arr = per_pair.reshape(KK, N_PAIRS, w)
    even = np.ascontiguousarray(arr[:, 0::2, :].reshape(KK, 64 * w))
    odd = np.ascontiguousarray(arr[:, 1::2, :].reshape(KK, 64 * w))
    return even, odd


def _host_wl(x_shard):
    """lhsT source: even/odd compact [12, 64*256] fp16 pair (see _pack2)."""
    u = PHI0 + x_shard.astype(np.float64)          # [256b, 256j]
    cu = -BC * np.cos(u)
    su = BC * np.sin(u)
    cu_hi, cu_lo = _split16(cu)
    su_hi, su_lo = _split16(su)
    rows = np.stack([cu_hi, cu_hi, cu_lo, su_hi, su_hi, su_lo], 0)  # [6,b,j]
    t_even = rows[:, :, 0::2].transpose(0, 2, 1)   # [6, 128p, 256b]
    t_odd = rows[:, :, 1::2].transpose(0, 2, 1)
    T = np.stack([t_even, t_odd], 0)               # [2, 6, 128p, 256b]
    # [h(2), s(6), p, b] -> [6h+s, p*256+b]
    return _pack2(T.reshape(KK, N_PAIRS * 256), 256)


def _host_rr(phase_offset):
    """rhs source: even/odd compact [12, 64*512] fp16 pair, block-diagonal
    per j-pair (see _pack2)."""
    p = phase_offset.astype(np.float64)            # [256i, 256j]
    cp_hi, cp_lo = _split16(np.cos(p))
    sp_hi, sp_lo = _split16(np.sin(p))
    rows = np.stack([cp_hi, cp_lo, cp_hi, sp_hi, sp_lo, sp_hi], 0)  # [6, i, j]
    r_even = rows[:, :, 0::2].transpose(0, 2, 1)   # [6, 128p, 256i]
    r_odd = rows[:, :, 1::2].transpose(0, 2, 1)
    Z = np.zeros((2, 6, N_PAIRS, 2, 256), np.float16)      # [h, s, p, h', i]
    Z[0, :, :, 0, :] = r_even
    Z[1, :, :, 1, :] = r_odd
    return _pack2(Z.reshape(KK, N_PAIRS * 512), 512)


def _act_recip(nc, out, in_, bias):
    """out = 1/(in_ + bias) on ScalarE via direct InstActivation emission."""
    eng = nc.scalar
    inputs = [eng.lower_ap(in_)]
    for arg in [bias, 1.0, 0.0]:  # bias, scale, alpha
        inputs.append(mybir.ImmediateValue(dtype=mybir.dt.float32, value=float(arg)))
    return eng.add_instruction(
        mybir.InstActivation(
            name=nc.get_next_instruction_name(),
            func=mybir.ActivationFunctionType.Reciprocal,
            ins=inputs,
            outs=[eng.lower_ap(out)],
        )
    )


# which engine accumulates group q (within a bt): True -> GpSimd, else Vector.
# GP is ~2x slower per op than DVE: give it every 3rd group with slack, none
# near the bt end so it can pre-fold its accumulator off the critical tail.
def _is_gp(q):
    return q in (1, 4, 6, 9, 11, 14, 16, 19, 21, 24, 26, 28)


N_INJ = 0                                         # trailing groups TensorE injects
_IS_INJ = lambda q: q >= GQ - N_INJ


def _is_dv(q):
    return not _is_gp(q) and not _IS_INJ(q)


_N_GP = sum(1 for q in range(GQ) if _is_gp(q))    # GP groups per bt (11)
_N_DV = sum(1 for q in range(GQ) if _is_dv(q))    # DVE groups per bt (19)
_GP_OPS_BT = _N_GP + 2                            # adds + 2 pre-folds


def build_nc(n_reps=1):
    """Build the per-core Bass graph. n_reps>1 repeats compute for timing."""
    f16, f32 = mybir.dt.float16, mybir.dt.float32
    add, mult = mybir.AluOpType.add, mybir.AluOpType.mult

    f32r = mybir.dt.float32r
    nc = bass.Bass("TRN2", target_bir_lowering=False, debug=False)
    wle = nc.dram_tensor("WLE", [KK, 64 * 256], f16, kind="ExternalInput").ap()
    wlo = nc.dram_tensor("WLO", [KK, 64 * 256], f16, kind="ExternalInput").ap()
    rre = nc.dram_tensor("RRE", [KK, 64 * 512], f16, kind="ExternalInput").ap()
    rro = nc.dram_tensor("RRO", [KK, 64 * 512], f16, kind="ExternalInput").ap()
    idm = nc.dram_tensor("IDM", [128, 128], f32, kind="ExternalInput").ap()
    y = nc.dram_tensor("OUT", [BL, 256], f32, kind="ExternalOutput").ap()

    # input DMA chunk plan over 64 col-blocks: tiny first chunks so the first
    # matmuls (and hence ScalarE) start as early as possible.
    # chunk 0 (first 2 col-blocks) goes out on three queues in parallel
    # (sync x2 + scalar + gpsimd, tracked on dma0_sem); the remaining chunks
    # stream in-order on sync's queue with growing sizes.
    CHUNK_PLAN = [2, 4, 8, 16, 34]
    assert sum(CHUNK_PLAN) == 64
    IN_DMAS_SYNC = 2 + 4 * (len(CHUNK_PLAN) - 1) + 1  # dma_sem incs from sync (+IDM)
    # cb -> dma_sem value once its (sync) chunk fully landed
    _cb_full_v = {}
    acc = CHUNK_PLAN[0]
    v = 32                      # sync's two chunk-0 DMAs
    for n_cb in CHUNK_PLAN[1:]:
        v += 64
        for cb in range(acc, acc + n_cb):
            _cb_full_v[cb] = v
        acc += n_cb

    def _wl_slice(p, bt):
        base = 64 * (p % 2)
        col = (p // 2) * 256 + bt * 128
        return (base, col)

    FD = PPG * 512   # 2048 free elems per group

    from contextlib import ExitStack
    with ExitStack() as st:
        ec = st.enter_context
        wls = ec(nc.sbuf_tensor("wls", [128, 64 * 256], f16))
        rrs = ec(nc.sbuf_tensor("rrs", [128, 64 * 512], f16))
        ids = ec(nc.sbuf_tensor("ids", [128, 128], f32))
        rt = [ec(nc.sbuf_tensor(f"rt{i}", [128, FD], f32)) for i in range(NRT)]
        s_dv = ec(nc.sbuf_tensor("s_dv", [128, FD], f32))
        s_gp = [ec(nc.sbuf_tensor(f"s_gp{i}", [128, FD], f32)) for i in range(2)]
        o_t = [ec(nc.sbuf_tensor(f"o{i}", [128, 256], f32)) for i in range(2)]
        dg = [ec(nc.psum_tensor(f"dg{i}", [128, FD], f32)) for i in range(NDG)]
        dma_sem = ec(nc.semaphore())
        dma0_sem = ec(nc.semaphore())  # chunk-0 DMAs on scalar/gpsimd queues
        mm_sem = ec(nc.semaphore())    # d-matmuls
        act_sem = ec(nc.semaphore())   # ACT recip groups
        dv_sem = ec(nc.semaphore())    # DVE chain adds
        gp_sem = ec(nc.semaphore())    # GP chain adds
        inj_sem = ec(nc.semaphore())   # TensorE tail injects
        sp_sem = ec(nc.semaphore())    # DVE consumed tail-inject PSUM
        o_sem = ec(nc.semaphore())     # per-bt finished outputs
        blk = ec(nc.Block())

        # ---- static schedules / sem count maps ----
        # global group index: gg = (rep*2 + bt)*GQ + q
        n_bt = 2 * n_reps

        def gg_of(rep, bt, q):
            return (rep * 2 + bt) * GQ + q

        # chain-consumer sem value after the accumulate op for global group g.
        # GP emits _GP_OPS_BT ops per bt (adds + 2 pre-folds); DVE emits _N_DV
        # chain adds per bt (tail fold/merge ops do not inc dv_sem); the last
        # N_INJ groups are injected by TensorE (4 inj_sem incs per group).
        cons = {}   # g -> (sem_name, value)
        for g in range(n_bt * GQ):
            blk_i, q = divmod(g, GQ)
            if _is_gp(q):
                cons[g] = ("gp", blk_i * _GP_OPS_BT + sum(
                    1 for qq in range(q + 1) if _is_gp(qq)))
            elif _IS_INJ(q):
                cons[g] = ("inj", blk_i * 4 * N_INJ
                           + 4 * (q - (GQ - N_INJ) + 1))
            else:
                cons[g] = ("dv", blk_i * _N_DV + sum(
                    1 for qq in range(q + 1) if _is_dv(qq)))

        # chunk 0's four DMAs issue from four different engines' queues so
        # they land in parallel (shortest path to the first matmul); sync
        # then waits for them before streaming the remaining chunks in-order
        # (keeps the cumulative dma_sem thresholds valid).
        def _chunk_slices(acc, n_cb):
            lo, hi = acc * 256, (acc + n_cb) * 256
            lo2, hi2 = acc * 512, (acc + n_cb) * 512
            return [(wls[0:KK, lo:hi], wle[:, lo:hi]),
                    (rrs[0:KK, lo2:hi2], rre[:, lo2:hi2]),
                    (wls[64:64 + KK, lo:hi], wlo[:, lo:hi]),
                    (rrs[64:64 + KK, lo2:hi2], rro[:, lo2:hi2])]

        @blk.sync
        def _(sync):
            c0 = _chunk_slices(0, CHUNK_PLAN[0])
            sync.dma_start(c0[0][0], c0[0][1]).then_inc(dma_sem, 16)
            sync.dma_start(c0[3][0], c0[3][1]).then_inc(dma_sem, 16)
            acc = CHUNK_PLAN[0]
            for n_cb in CHUNK_PLAN[1:]:
                for dst, src in _chunk_slices(acc, n_cb):
                    sync.dma_start(dst, src).then_inc(dma_sem, 16)
                acc += n_cb
            sync.dma_start(ids[:], idm[:]).then_inc(dma_sem, 16)
            for rep in range(n_reps):
                for bt in range(2):
                    sync.wait_ge(o_sem, rep * 2 + bt + 1)
                    sync.dma_start(y[bt * 128:(bt + 1) * 128, :], o_t[bt][:]
                                   ).then_inc(dma_sem, 16)

        @blk.tensor
        def _(tensor):
            for rep in range(n_reps):
                for bt in range(2):
                    blk_i = rep * 2 + bt
                    for q in range(GQ):
                        gg = gg_of(rep, bt, q)
                        if gg >= NDG:
                            # ACT must have consumed dg[gg % NDG]
                            tensor.wait_ge(act_sem, gg - NDG + 1)
                        if q == 0 and blk_i > 0 and N_INJ:
                            # tail-inject region (dg[even][:, 0:512]) of the
                            # previous bt must have been read by DVE
                            tensor.wait_ge(sp_sem, blk_i)
                        for h in range(PPG):
                            p = q * PPG + h
                            if rep == 0 and bt == 0:
                                cb = p // 2
                                if cb < CHUNK_PLAN[0]:
                                    tensor.wait_ge(dma_sem, 32)
                                    tensor.wait_ge(dma0_sem, 32)
                                else:
                                    tensor.wait_ge(dma_sem, _cb_full_v[cb])
                            base, col = _wl_slice(p, bt)
                            tensor.matmul(
                                dg[gg % NDG][:, h * 512:(h + 1) * 512],
                                wls[base:base + KK, col:col + 128],
                                rrs[base:base + KK, (p // 2) * 512:(p // 2) * 512 + 512],
                                start=True, stop=True,
                            ).then_inc(mm_sem, 1)
                    # tail injects: accumulate the last N_INJ groups' r-tiles
                    # into dg[even][:, 0:512] (one PSUM bank) via float32r
                    # identity matmuls -- folds 2048 -> 512 for free.
                    for k in range(N_INJ):
                        q = GQ - N_INJ + k
                        gg = gg_of(rep, bt, q)
                        s_ps = dg[gg_of(rep, bt, GQ - N_INJ) % NDG]
                        tensor.wait_ge(act_sem, gg + 1)
                        for c in range(4):
                            tensor.matmul(
                                s_ps[:, 0:512],
                                ids[:].bitcast(f32r),
                                rt[gg % NRT][:, c * 512:(c + 1) * 512].bitcast(f32r),
                                start=(k == 0 and c == 0),
                                stop=(k == N_INJ - 1 and c == 3),
                                skip_group_check=True,
                            ).then_inc(inj_sem, 1)

        @blk.scalar
        def _(scalar):
            dst, src = _chunk_slices(0, CHUNK_PLAN[0])[1]
            scalar.dma_start(dst, src).then_inc(dma0_sem, 16)
            sems = {"gp": gp_sem, "dv": dv_sem, "inj": inj_sem}
            for rep in range(n_reps):
                for bt in range(2):
                    for q in range(GQ):
                        gg = gg_of(rep, bt, q)
                        if gg >= NRT:
                            # accumulation chain must have consumed rt[gg % NRT]
                            sem_name, val = cons[gg - NRT]
                            scalar.wait_ge(sems[sem_name], val)
                        if gg == 0:
                            # split the very first group so ScalarE starts
                            # after only 2 of its 4 matmuls have landed
                            scalar.wait_ge(mm_sem, 2)
                            _act_recip(nc, rt[0][:, 0:1024], dg[0][:, 0:1024],
                                       D_CONST)
                            scalar.wait_ge(mm_sem, 4)
                            _act_recip(nc, rt[0][:, 1024:2048], dg[0][:, 1024:2048],
                                       D_CONST).then_inc(act_sem, 1)
                            continue
                        scalar.wait_ge(mm_sem, PPG * (gg + 1))
                        _act_recip(nc, rt[gg % NRT][:], dg[gg % NDG][:], D_CONST
                                   ).then_inc(act_sem, 1)

        @blk.gpsimd
        def _(gpsimd):
            dst, src = _chunk_slices(0, CHUNK_PLAN[0])[2]
            gpsimd.dma_start(dst, src).then_inc(dma0_sem, 16)
            for rep in range(n_reps):
                for bt in range(2):
                    sg = s_gp[bt]
                    if rep > 0:
                        # previous rep's merge for this bt parity consumed sg
                        gpsimd.wait_ge(o_sem, 2 * (rep - 1) + bt + 1)
                    first = True
                    for q in range(GQ):
                        if not _is_gp(q):
                            continue
                        gg = gg_of(rep, bt, q)
                        gpsimd.wait_ge(act_sem, gg + 1)
                        if first:
                            gpsimd.tensor_copy(sg[:], rt[gg % NRT][:]
                                               ).then_inc(gp_sem, 1)
                        else:
                            gpsimd.tensor_tensor(sg[:], sg[:], rt[gg % NRT][:],
                                                 add).then_inc(gp_sem, 1)
                        first = False
                    # pre-fold own accumulator 2048 -> 512 (off the tail path)
                    gpsimd.tensor_tensor(sg[:, 0:1024], sg[:, 0:1024],
                                         sg[:, 1024:2048], add).then_inc(gp_sem, 1)
                    gpsimd.tensor_tensor(sg[:, 0:512], sg[:, 0:512],
                                         sg[:, 512:1024], add).then_inc(gp_sem, 1)

        @blk.vector
        def _(vector):
            for rep in range(n_reps):
                for bt in range(2):
                    blk_i = rep * 2 + bt
                    first = True
                    for q in range(GQ):
                        if not _is_dv(q):
                            continue
                        gg = gg_of(rep, bt, q)
                        vector.wait_ge(act_sem, gg + 1)
                        if first:
                            vector.tensor_copy(s_dv[:], rt[gg % NRT][:]
                                               ).then_inc(dv_sem, 1)
                        else:
                            vector.tensor_tensor(s_dv[:], s_dv[:], rt[gg % NRT][:],
                                                 add).then_inc(dv_sem, 1)
                        first = False
                    # fold 2048 -> 512, merge GP, then the injected PSUM bank
                    vector.tensor_tensor(s_dv[:, 0:1024], s_dv[:, 0:1024],
                                         s_dv[:, 1024:2048], add)
                    vector.tensor_tensor(s_dv[:, 0:512], s_dv[:, 0:512],
                                         s_dv[:, 512:1024], add)
                    vector.wait_ge(gp_sem, (blk_i + 1) * _GP_OPS_BT)
                    vector.tensor_tensor(s_dv[:, 0:512], s_dv[:, 0:512],
                                         s_gp[bt][:, 0:512], add)
                    if N_INJ:
                        s_ps = dg[gg_of(rep, bt, GQ - N_INJ) % NDG]
                        vector.wait_ge(inj_sem, (blk_i + 1) * 4 * N_INJ)
                        vector.tensor_tensor(s_dv[:, 0:512], s_dv[:, 0:512],
                                             s_ps[:, 0:512], add).then_inc(sp_sem, 1)
                    if rep > 0:
                        vector.wait_ge(dma_sem,
                                       16 * (IN_DMAS_SYNC + 2 * (rep - 1) + bt + 1))
                    vector.tensor_tensor(o_t[bt][:], s_dv[:, 0:256], s_dv[:, 256:512],
                                         add)
                    vector.tensor_scalar(o_t[bt][:], o_t[bt][:],
                                         -GAMMA, float(I_DIM), mult, add
                                         ).then_inc(o_sem, 1)

    return nc


_NC_CACHE = {}


def _get_nc(n_reps=1):
    if n_reps not in _NC_CACHE:
        _NC_CACHE[n_reps] = build_nc(n_reps)
    return _NC_CACHE[n_reps]


def kernel(input_matrix, phase_offset, n_reps=1):
    input_matrix = np.asarray(input_matrix, np.float32)
    phase_offset = np.asarray(phase_offset, np.float32)
    rre, rro = _host_rr(phase_offset)
    idm = np.eye(128, dtype=np.float32)
    in_maps = []
    for c in range(N_CORES):
        wle, wlo = _host_wl(input_matrix[c * BL:(c + 1) * BL, :])
        in_maps.append({"WLE": wle, "WLO": wlo, "RRE": rre, "RRO": rro,
                        "IDM": idm})
    nc = _get_nc(n_reps)
    res = run_bass_kernel_spmd(nc, in_maps, core_ids=list(range(N_CORES)))
    out = np.empty((B, O_DIM), np.float32)
    for c in range(N_CORES):
        out[c * BL:(c + 1) * BL, :] = res.results[c]["OUT"]
    return out


# revision 88
# speedup vs baseline: 1.0218x; 1.0173x over previous
"""PhotonicCore micro-ring transmission kernel for 8x TRN2 NeuronCores.

Math:
    phi = phi0 + X[b,j] + P[i,j]
    trans = (t^2 - 2at*cos(phi) + a^2) / (1 - 2at*cos(phi) + (at)^2)
          = 1 - gamma / d,   d = D - Bc*cos(phi),  D = 1+(at)^2, Bc = 2at,
            gamma = kappa*(1-a^2)
    out[b,i] = sum_j trans = 256 - gamma * sum_j 1/d[b,i,j]

Device strategy (data-parallel over batch, 256 rows/core):
    cos(phi) = cos(u)cos(p) - sin(u)sin(p)  with u = phi0 + X  ==>  for a
    fixed j, -Bc*cos(phi)[b,i] is a rank-2 outer product.  TensorE matmuls
    (K=12, fp16 hi/lo split => ~fp32-exact products) build -Bc*cos(phi)
    for 8 j-columns into a 4-bank PSUM group [128b x 2048].  ScalarE does
    the one mandatory per-element pass r = Reciprocal(psum + D) (measured
    2.7e-5 max rel err on our d range) into SBUF; the j-reduction runs as
    two independent accumulation chains on VectorE and GpSimdE (S += r),
    merged + folded + scaled by VectorE per 128-row block.
"""

import numpy as np

import concourse.bass as bass
import concourse.mybir as mybir
from concourse.bass_utils import run_bass_kernel_spmd

# ---- problem constants (hardcoded; kernel.py must be self-contained) ----
B, I_DIM, O_DIM = 2048, 256, 256
N_CORES = 8
BL = B // N_CORES  # 256 batch rows per core

KAPPA = 0.1
LOSS_A = 0.99
N_EFF = 3.48
LAMBDA = 1.55e-6
RADIUS = 5e-6

_t = np.sqrt(1.0 - KAPPA)
_a = LOSS_A
_x = _a * _t
BC = 2.0 * _a * _t
D_CONST = 1.0 + _x * _x
GAMMA = KAPPA * (1.0 - _a * _a)
PHI0 = float(np.float32((2.0 * np.pi * N_EFF * (2.0 * np.pi * RADIUS) / LAMBDA) % (2.0 * np.pi)))

N_PAIRS = 128   # j pairs per core (256 j / 2)
KK = 12         # contraction rows per matmul (6 per j)
PPG = 4         # pairs per PSUM group (4 pairs = 8 j = [128, 2048] = 4 banks)
GQ = N_PAIRS // PPG          # 32 groups per bt
NRT = 6         # r-tile rotation depth
NDG = 2         # PSUM d-group rotation depth


def _split16(v):
    hi = v.astype(np.float16)
    lo = (v - hi.astype(np.float64)).astype(np.float16)
    return hi, lo


def _pack2(per_pair, w):
    """Split per-pair [12, w] blocks into even/odd-pair compact tensors
    [12, 64, w] (DRAM stays compact; the DMA scatters even pairs to SBUF
    partition base 0 and odd pairs to base 64, where matmul can slice
    [12, w] blocks at legal partition bases)."""
    arr = per_pair.reshape(KK, N_PAIRS, w)
    return arr[:, 0::2, :], arr[:, 1::2, :]


def _host_wl(x_shard):
    """lhsT source: even/odd compact [12, 64*256] fp16 pair (see _pack2)."""
    u = PHI0 + x_shard.astype(np.float64)          # [256b, 256j]
    cu = -BC * np.cos(u)
    su = BC * np.sin(u)
    cu_hi, cu_lo = _split16(cu)
    su_hi, su_lo = _split16(su)
    rows = np.stack([cu_hi, cu_hi, cu_lo, su_hi, su_hi, su_lo], 0)  # [6,b,j]
    t_even = rows[:, :, 0::2].transpose(0, 2, 1)   # [6, 128p, 256b]
    t_odd = rows[:, :, 1::2].transpose(0, 2, 1)
    T = np.stack([t_even, t_odd], 0)               # [2, 6, 128p, 256b]
    # [h(2), s(6), p, b] -> [6h+s, p*256+b]
    return _pack2(T.reshape(KK, N_PAIRS * 256), 256)  # 2x [12, 64, 256]


def _host_wr(x_shard, rr_pair):
    """Combined per-parity operand tensors [12, 64*768] fp16: per col-block
    cb the layout is [wl(cb) 256 cols | rr(cb) 512 cols], so each input DMA
    chunk is ONE transfer per parity and matmul slices stay contiguous."""
    wl_e, wl_o = _host_wl(x_shard)
    rr_e, rr_o = rr_pair
    we = np.concatenate([wl_e, rr_e], axis=2).reshape(KK, 64 * 768)
    wo = np.concatenate([wl_o, rr_o], axis=2).reshape(KK, 64 * 768)
    return np.ascontiguousarray(we), np.ascontiguousarray(wo)


def _host_rr(phase_offset):
    """rhs source: even/odd compact [12, 64*512] fp16 pair, block-diagonal
    per j-pair (see _pack2)."""
    p = phase_offset.astype(np.float64)            # [256i, 256j]
    cp_hi, cp_lo = _split16(np.cos(p))
    sp_hi, sp_lo = _split16(np.sin(p))
    rows = np.stack([cp_hi, cp_lo, cp_hi, sp_hi, sp_lo, sp_hi], 0)  # [6, i, j]
    r_even = rows[:, :, 0::2].transpose(0, 2, 1)   # [6, 128p, 256i]
    r_odd = rows[:, :, 1::2].transpose(0, 2, 1)
    Z = np.zeros((2, 6, N_PAIRS, 2, 256), np.float16)      # [h, s, p, h', i]
    Z[0, :, :, 0, :] = r_even
    Z[1, :, :, 1, :] = r_odd
    return _pack2(Z.reshape(KK, N_PAIRS * 512), 512)  # 2x [12, 64, 512]


def _act_recip(nc, out, in_, bias):
    """out = 1/(in_ + bias) on ScalarE via direct InstActivation emission."""
    eng = nc.scalar
    inputs = [eng.lower_ap(in_)]
    for arg in [bias, 1.0, 0.0]:  # bias, scale, alpha
        inputs.append(mybir.ImmediateValue(dtype=mybir.dt.float32, value=float(arg)))
    return eng.add_instruction(
        mybir.InstActivation(
            name=nc.get_next_instruction_name(),
            func=mybir.ActivationFunctionType.Reciprocal,
            ins=inputs,
            outs=[eng.lower_ap(out)],
        )
    )


# which engine accumulates group q (within a bt): True -> GpSimd, else Vector.
# GP is ~2x slower per op than DVE: give it every 3rd group with slack, none
# near the bt end so it can pre-fold its accumulator off the critical tail.
def _is_gp(q):
    return q in (1, 4, 6, 9, 11, 14, 16, 19, 21, 24, 26, 28)


def _is_dv(q):
    return not _is_gp(q)


_N_GP = sum(1 for q in range(GQ) if _is_gp(q))    # GP groups per bt (11)
_N_DV = sum(1 for q in range(GQ) if _is_dv(q))    # DVE groups per bt (19)
_GP_OPS_BT = _N_GP + 2                            # adds + 2 pre-folds


def build_nc(n_reps=1):
    """Build the per-core Bass graph. n_reps>1 repeats compute for timing."""
    f16, f32 = mybir.dt.float16, mybir.dt.float32
    add, mult = mybir.AluOpType.add, mybir.AluOpType.mult

    nc = bass.Bass("TRN2", target_bir_lowering=False, debug=False)
    wre = nc.dram_tensor("WRE", [KK, 64 * 768], f16, kind="ExternalInput").ap()
    wro = nc.dram_tensor("WRO", [KK, 64 * 768], f16, kind="ExternalInput").ap()
    y = nc.dram_tensor("OUT", [BL, 256], f32, kind="ExternalOutput").ap()

    # input DMA chunk plan over 64 col-blocks: one combined transfer per
    # parity per chunk.  Chunk 0's two DMAs go out in parallel on sync's and
    # scalar's queues (tracked on dma0_sem for the odd one); the remaining
    # chunks stream in-order on sync's queue with growing sizes.
    CHUNK_PLAN = [2, 4, 8, 16, 34]
    assert sum(CHUNK_PLAN) == 64
    IN_DMAS_SYNC = 1 + 2 * (len(CHUNK_PLAN) - 1)  # dma_sem incs from sync
    # cb -> dma_sem value once its (sync) chunk fully landed
    _cb_full_v = {}
    acc = CHUNK_PLAN[0]
    v = 16                      # sync's chunk-0 DMA
    for n_cb in CHUNK_PLAN[1:]:
        v += 32
        for cb in range(acc, acc + n_cb):
            _cb_full_v[cb] = v
        acc += n_cb

    FD = PPG * 512   # 2048 free elems per group

    from contextlib import ExitStack
    with ExitStack() as st:
        ec = st.enter_context
        wrs = ec(nc.sbuf_tensor("wrs", [128, 64 * 768], f16))
        rt = [ec(nc.sbuf_tensor(f"rt{i}", [128, FD], f32)) for i in range(NRT)]
        s_dv = ec(nc.sbuf_tensor("s_dv", [128, FD], f32))
        s_gp = [ec(nc.sbuf_tensor(f"s_gp{i}", [128, FD], f32)) for i in range(2)]
        o_t = [ec(nc.sbuf_tensor(f"o{i}", [128, 256], f32)) for i in range(2)]
        dg = [ec(nc.psum_tensor(f"dg{i}", [128, FD], f32)) for i in range(NDG)]
        dma_sem = ec(nc.semaphore())
        dma0_sem = ec(nc.semaphore())  # chunk-0 odd DMA on scalar's queue
        mm_sem = ec(nc.semaphore())    # d-matmuls
        act_sem = ec(nc.semaphore())   # ACT recip groups
        dv_sem = ec(nc.semaphore())    # DVE chain adds
        gp_sem = ec(nc.semaphore())    # GP chain adds
        o_sem = ec(nc.semaphore())     # per-bt finished outputs
        blk = ec(nc.Block())

        # ---- static schedules / sem count maps ----
        # global group index: gg = (rep*2 + bt)*GQ + q
        n_bt = 2 * n_reps

        def gg_of(rep, bt, q):
            return (rep * 2 + bt) * GQ + q

        # chain-consumer sem value after the accumulate op for global group g.
        # GP emits _GP_OPS_BT ops per bt (adds + 2 pre-folds); DVE emits _N_DV
        # chain adds per bt (tail fold/merge ops do not inc dv_sem).
        cons = {}   # g -> (sem_name, value)
        for g in range(n_bt * GQ):
            blk_i, q = divmod(g, GQ)
            if _is_gp(q):
                cons[g] = ("gp", blk_i * _GP_OPS_BT + sum(
                    1 for qq in range(q + 1) if _is_gp(qq)))
            else:
                cons[g] = ("dv", blk_i * _N_DV + sum(
                    1 for qq in range(q + 1) if _is_dv(qq)))

        # one combined DMA per (parity, chunk): [even -> partitions 0:12,
        # odd -> 64:76].  Chunk 0's two DMAs run in parallel (sync + scalar).
        def _chunk_slices(acc, n_cb):
            lo, hi = acc * 768, (acc + n_cb) * 768
            return [(wrs[0:KK, lo:hi], wre[:, lo:hi]),
                    (wrs[64:64 + KK, lo:hi], wro[:, lo:hi])]

        @blk.sync
        def _(sync):
            c0 = _chunk_slices(0, CHUNK_PLAN[0])
            sync.dma_start(c0[0][0], c0[0][1]).then_inc(dma_sem, 16)
            acc = CHUNK_PLAN[0]
            for n_cb in CHUNK_PLAN[1:]:
                for dst, src in _chunk_slices(acc, n_cb):
                    sync.dma_start(dst, src).then_inc(dma_sem, 16)
                acc += n_cb
            for rep in range(n_reps):
                for bt in range(2):
                    sync.wait_ge(o_sem, rep * 2 + bt + 1)
                    sync.dma_start(y[bt * 128:(bt + 1) * 128, :], o_t[bt][:]
                                   ).then_inc(dma_sem, 16)

        @blk.tensor
        def _(tensor):
            # warm-up matmul (ungated, result overwritten by q0's start=True):
            # starts the PE p-state ramp before the first DMA lands
            tensor.matmul(dg[1][:, 0:512], wrs[0:KK, 0:128],
                          wrs[0:KK, 256:768], start=True, stop=True)
            for rep in range(n_reps):
                for bt in range(2):
                    blk_i = rep * 2 + bt
                    for q in range(GQ):
                        gg = gg_of(rep, bt, q)
                        if gg >= NDG:
                            # ACT must have consumed dg[gg % NDG]
                            tensor.wait_ge(act_sem, gg - NDG + 1)
                        for h in range(PPG):
                            p = q * PPG + h
                            if rep == 0 and bt == 0:
                                cb = p // 2
                                if cb < CHUNK_PLAN[0]:
                                    tensor.wait_ge(dma_sem, 16)
                                    tensor.wait_ge(dma0_sem, 16)
                                else:
                                    tensor.wait_ge(dma_sem, _cb_full_v[cb])
                            base = 64 * (p % 2)
                            col = (p // 2) * 768
                            tensor.matmul(
                                dg[gg % NDG][:, h * 512:(h + 1) * 512],
                                wrs[base:base + KK, col + bt * 128:col + bt * 128 + 128],
                                wrs[base:base + KK, col + 256:col + 768],
                                start=True, stop=True,
                            ).then_inc(mm_sem, 1)

        @blk.scalar
        def _(scalar):
            dst, src = _chunk_slices(0, CHUNK_PLAN[0])[1]
            scalar.dma_start(dst, src).then_inc(dma0_sem, 16)
            sems = {"gp": gp_sem, "dv": dv_sem}
            for rep in range(n_reps):
                for bt in range(2):
                    for q in range(GQ):
                        gg = gg_of(rep, bt, q)
                        if gg >= NRT:
                            # accumulation chain must have consumed rt[gg % NRT]
                            sem_name, val = cons[gg - NRT]
                            scalar.wait_ge(sems[sem_name], val)
                        if gg == 0:
                            # split the very first group so ScalarE starts
                            # after only 2 of its 4 matmuls have landed
                            scalar.wait_ge(mm_sem, 2)
                            _act_recip(nc, rt[0][:, 0:1024], dg[0][:, 0:1024],
                                       D_CONST)
                            scalar.wait_ge(mm_sem, 4)
                            _act_recip(nc, rt[0][:, 1024:2048], dg[0][:, 1024:2048],
                                       D_CONST).then_inc(act_sem, 1)
                            continue
                        scalar.wait_ge(mm_sem, PPG * (gg + 1))
                        _act_recip(nc, rt[gg % NRT][:], dg[gg % NDG][:], D_CONST
                                   ).then_inc(act_sem, 1)

        @blk.gpsimd
        def _(gpsimd):
            for rep in range(n_reps):
                for bt in range(2):
                    sg = s_gp[bt]
                    if rep > 0:
                        # previous rep's merge for this bt parity consumed sg
                        gpsimd.wait_ge(o_sem, 2 * (rep - 1) + bt + 1)
                    first = True
                    for q in range(GQ):
                        if not _is_gp(q):
                            continue
                        gg = gg_of(rep, bt, q)
                        gpsimd.wait_ge(act_sem, gg + 1)
                        if first:
                            gpsimd.tensor_copy(sg[:], rt[gg % NRT][:]
                                               ).then_inc(gp_sem, 1)
                        else:
                            gpsimd.tensor_tensor(sg[:], sg[:], rt[gg % NRT][:],
                                                 add).then_inc(gp_sem, 1)
                        first = False
                    # pre-fold own accumulator 2048 -> 512 (off the tail path)
                    gpsimd.tensor_tensor(sg[:, 0:1024], sg[:, 0:1024],
                                         sg[:, 1024:2048], add).then_inc(gp_sem, 1)
                    gpsimd.tensor_tensor(sg[:, 0:512], sg[:, 0:512],
                                         sg[:, 512:1024], add).then_inc(gp_sem, 1)

        @blk.vector
        def _(vector):
            for rep in range(n_reps):
                for bt in range(2):
                    blk_i = rep * 2 + bt
                    first = True
                    for q in range(GQ):
                        if not _is_dv(q):
                            continue
                        gg = gg_of(rep, bt, q)
                        vector.wait_ge(act_sem, gg + 1)
                        if first:
                            vector.tensor_copy(s_dv[:], rt[gg % NRT][:]
                                               ).then_inc(dv_sem, 1)
                        else:
                            vector.tensor_tensor(s_dv[:], s_dv[:], rt[gg % NRT][:],
                                                 add).then_inc(dv_sem, 1)
                        first = False
                    # fold 2048 -> 512, merge GP's pre-folded accumulator
                    vector.tensor_tensor(s_dv[:, 0:1024], s_dv[:, 0:1024],
                                         s_dv[:, 1024:2048], add)
                    vector.tensor_tensor(s_dv[:, 0:512], s_dv[:, 0:512],
                                         s_dv[:, 512:1024], add)
                    vector.wait_ge(gp_sem, (blk_i + 1) * _GP_OPS_BT)
                    vector.tensor_tensor(s_dv[:, 0:512], s_dv[:, 0:512],
                                         s_gp[bt][:, 0:512], add)
                    if rep > 0:
                        vector.wait_ge(dma_sem,
                                       16 * (IN_DMAS_SYNC + 2 * (rep - 1) + bt + 1))
                    vector.tensor_tensor(o_t[bt][:], s_dv[:, 0:256], s_dv[:, 256:512],
                                         add)
                    vector.tensor_scalar(o_t[bt][:], o_t[bt][:],
                                         -GAMMA, float(I_DIM), mult, add
                                         ).then_inc(o_sem, 1)

    return nc


_NC_CACHE = {}


def _get_nc(n_reps=1):
    if n_reps not in _NC_CACHE:
        _NC_CACHE[n_reps] = build_nc(n_reps)
    return _NC_CACHE[n_reps]


def kernel(input_matrix, phase_offset, n_reps=1):
    input_matrix = np.asarray(input_matrix, np.float32)
    phase_offset = np.asarray(phase_offset, np.float32)
    rr_pair = _host_rr(phase_offset)
    in_maps = []
    for c in range(N_CORES):
        we, wo = _host_wr(input_matrix[c * BL:(c + 1) * BL, :], rr_pair)
        in_maps.append({"WRE": we, "WRO": wo})
    nc = _get_nc(n_reps)
    res = run_bass_kernel_spmd(nc, in_maps, core_ids=list(range(N_CORES)))
    out = np.empty((B, O_DIM), np.float32)
    for c in range(N_CORES):
        out[c * BL:(c + 1) * BL, :] = res.results[c]["OUT"]
    return out


# revision 107
# speedup vs baseline: 1.0238x; 1.0020x over previous
"""PhotonicCore micro-ring transmission kernel for 8x TRN2 NeuronCores.

Math:
    phi = phi0 + X[b,j] + P[i,j]
    trans = (t^2 - 2at*cos(phi) + a^2) / (1 - 2at*cos(phi) + (at)^2)
          = 1 - gamma / d,   d = D - Bc*cos(phi),  D = 1+(at)^2, Bc = 2at,
            gamma = kappa*(1-a^2)
    out[b,i] = sum_j trans = 256 - gamma * sum_j 1/d[b,i,j]

Device strategy (data-parallel over batch, 256 rows/core):
    cos(phi) = cos(u)cos(p) - sin(u)sin(p)  with u = phi0 + X  ==>  for a
    fixed j, -Bc*cos(phi)[b,i] is a rank-2 outer product.  TensorE matmuls
    (K=12, fp16 hi/lo split => ~fp32-exact products) build -Bc*cos(phi)
    for 8 j-columns into a 4-bank PSUM group [128b x 2048].  ScalarE does
    the one mandatory per-element pass r = Reciprocal(psum + D) (measured
    2.7e-5 max rel err on our d range) into SBUF; the j-reduction runs as
    two independent accumulation chains on VectorE and GpSimdE (S += r),
    merged + folded + scaled by VectorE per 128-row block.
"""

import numpy as np

import concourse.bass as bass
import concourse.mybir as mybir
from concourse.bass_utils import run_bass_kernel_spmd

# ---- problem constants (hardcoded; kernel.py must be self-contained) ----
B, I_DIM, O_DIM = 2048, 256, 256
N_CORES = 8
BL = B // N_CORES  # 256 batch rows per core

KAPPA = 0.1
LOSS_A = 0.99
N_EFF = 3.48
LAMBDA = 1.55e-6
RADIUS = 5e-6

_t = np.sqrt(1.0 - KAPPA)
_a = LOSS_A
_x = _a * _t
BC = 2.0 * _a * _t
D_CONST = 1.0 + _x * _x
GAMMA = KAPPA * (1.0 - _a * _a)
PHI0 = float(np.float32((2.0 * np.pi * N_EFF * (2.0 * np.pi * RADIUS) / LAMBDA) % (2.0 * np.pi)))

N_PAIRS = 128   # j pairs per core (256 j / 2)
KK = 12         # contraction rows per matmul (6 per j)
PPG = 4         # pairs per PSUM group (4 pairs = 8 j = [128, 2048] = 4 banks)
GQ = N_PAIRS // PPG          # 32 groups per bt
NRT = 6         # r-tile rotation depth
NDG = 2         # PSUM d-group rotation depth


def _split16(v):
    hi = v.astype(np.float16)
    lo = (v - hi.astype(np.float64)).astype(np.float16)
    return hi, lo


def _pack2(per_pair, w):
    """Split per-pair [12, w] blocks into even/odd-pair compact tensors
    [12, 64, w] (DRAM stays compact; the DMA scatters even pairs to SBUF
    partition base 0 and odd pairs to base 64, where matmul can slice
    [12, w] blocks at legal partition bases)."""
    arr = per_pair.reshape(KK, N_PAIRS, w)
    return arr[:, 0::2, :], arr[:, 1::2, :]


def _host_wl(x_shard):
    """lhsT source: even/odd compact [12, 64*256] fp16 pair (see _pack2)."""
    u = PHI0 + x_shard.astype(np.float64)          # [256b, 256j]
    cu = -BC * np.cos(u)
    su = BC * np.sin(u)
    cu_hi, cu_lo = _split16(cu)
    su_hi, su_lo = _split16(su)
    rows = np.stack([cu_hi, cu_hi, cu_lo, su_hi, su_hi, su_lo], 0)  # [6,b,j]
    t_even = rows[:, :, 0::2].transpose(0, 2, 1)   # [6, 128p, 256b]
    t_odd = rows[:, :, 1::2].transpose(0, 2, 1)
    T = np.stack([t_even, t_odd], 0)               # [2, 6, 128p, 256b]
    # [h(2), s(6), p, b] -> [6h+s, p*256+b]
    return _pack2(T.reshape(KK, N_PAIRS * 256), 256)  # 2x [12, 64, 256]


def _host_wr(x_shard, rr_pair):
    """Combined per-parity operand tensors [12, 64*768] fp16: per col-block
    cb the layout is [wl(cb) 256 cols | rr(cb) 512 cols], so each input DMA
    chunk is ONE transfer per parity and matmul slices stay contiguous."""
    wl_e, wl_o = _host_wl(x_shard)
    rr_e, rr_o = rr_pair
    we = np.concatenate([wl_e, rr_e], axis=2).reshape(KK, 64 * 768)
    wo = np.concatenate([wl_o, rr_o], axis=2).reshape(KK, 64 * 768)
    return np.ascontiguousarray(we), np.ascontiguousarray(wo)


def _host_rr(phase_offset):
    """rhs source: even/odd compact [12, 64*512] fp16 pair, block-diagonal
    per j-pair (see _pack2)."""
    p = phase_offset.astype(np.float64)            # [256i, 256j]
    cp_hi, cp_lo = _split16(np.cos(p))
    sp_hi, sp_lo = _split16(np.sin(p))
    rows = np.stack([cp_hi, cp_lo, cp_hi, sp_hi, sp_lo, sp_hi], 0)  # [6, i, j]
    r_even = rows[:, :, 0::2].transpose(0, 2, 1)   # [6, 128p, 256i]
    r_odd = rows[:, :, 1::2].transpose(0, 2, 1)
    Z = np.zeros((2, 6, N_PAIRS, 2, 256), np.float16)      # [h, s, p, h', i]
    Z[0, :, :, 0, :] = r_even
    Z[1, :, :, 1, :] = r_odd
    return _pack2(Z.reshape(KK, N_PAIRS * 512), 512)  # 2x [12, 64, 512]


def _act_recip(nc, out, in_, bias):
    """out = 1/(in_ + bias) on ScalarE via direct InstActivation emission."""
    eng = nc.scalar
    inputs = [eng.lower_ap(in_)]
    for arg in [bias, 1.0, 0.0]:  # bias, scale, alpha
        inputs.append(mybir.ImmediateValue(dtype=mybir.dt.float32, value=float(arg)))
    return eng.add_instruction(
        mybir.InstActivation(
            name=nc.get_next_instruction_name(),
            func=mybir.ActivationFunctionType.Reciprocal,
            ins=inputs,
            outs=[eng.lower_ap(out)],
        )
    )


# which engine accumulates group q (within a bt): True -> GpSimd, else Vector.
# GP is ~2x slower per op than DVE: give it every 3rd group with slack, none
# near the bt end so it can pre-fold its accumulator off the critical tail.
def _is_gp(q):
    return q in (1, 4, 6, 9, 11, 14, 16, 19, 21, 24, 26, 28)


def _is_dv(q):
    return not _is_gp(q)


_N_GP = sum(1 for q in range(GQ) if _is_gp(q))    # GP groups per bt (12)
_N_DV = sum(1 for q in range(GQ) if _is_dv(q))    # DVE groups per bt (20)
_GP_OPS_BT = _N_GP + 3                            # adds + 3 pre-folds (to 256)


def build_nc(n_reps=1):
    """Build the per-core Bass graph. n_reps>1 repeats compute for timing."""
    f16, f32 = mybir.dt.float16, mybir.dt.float32
    add, mult = mybir.AluOpType.add, mybir.AluOpType.mult

    nc = bass.Bass("TRN2", target_bir_lowering=False, debug=False)
    wre = nc.dram_tensor("WRE", [KK, 64 * 768], f16, kind="ExternalInput").ap()
    wro = nc.dram_tensor("WRO", [KK, 64 * 768], f16, kind="ExternalInput").ap()
    y = nc.dram_tensor("OUT", [BL, 256], f32, kind="ExternalOutput").ap()

    # input DMA chunk plan over 64 col-blocks: one combined transfer per
    # parity per chunk.  Chunk 0's two DMAs go out in parallel on sync's and
    # scalar's queues (tracked on dma0_sem for the odd one); the remaining
    # chunks stream in-order on sync's queue with growing sizes.
    CHUNK_PLAN = [2, 4, 8, 16, 34]
    assert sum(CHUNK_PLAN) == 64
    IN_DMAS_SYNC = 1 + 2 * (len(CHUNK_PLAN) - 1)  # dma_sem incs from sync
    # cb -> dma_sem value once its (sync) chunk fully landed
    _cb_full_v = {}
    acc = CHUNK_PLAN[0]
    v = 16                      # sync's chunk-0 DMA
    for n_cb in CHUNK_PLAN[1:]:
        v += 32
        for cb in range(acc, acc + n_cb):
            _cb_full_v[cb] = v
        acc += n_cb

    FD = PPG * 512   # 2048 free elems per group

    from contextlib import ExitStack
    with ExitStack() as st:
        ec = st.enter_context
        wrs = ec(nc.sbuf_tensor("wrs", [128, 64 * 768], f16))
        rt = [ec(nc.sbuf_tensor(f"rt{i}", [128, FD], f32)) for i in range(NRT)]
        s_dv = ec(nc.sbuf_tensor("s_dv", [128, FD], f32))
        s_gp = [ec(nc.sbuf_tensor(f"s_gp{i}", [128, FD], f32)) for i in range(2)]
        o_t = [ec(nc.sbuf_tensor(f"o{i}", [128, 256], f32)) for i in range(2)]
        dg = [ec(nc.psum_tensor(f"dg{i}", [128, FD], f32)) for i in range(NDG)]
        dma_sem = ec(nc.semaphore())
        dma0_sem = ec(nc.semaphore())  # chunk-0 odd DMA on scalar's queue
        mm_sem = ec(nc.semaphore())    # d-matmuls
        act_sem = ec(nc.semaphore())   # ACT recip groups
        dv_sem = ec(nc.semaphore())    # DVE chain adds
        gp_sem = ec(nc.semaphore())    # GP chain adds
        o_sem = ec(nc.semaphore())     # per-bt finished outputs
        blk = ec(nc.Block())

        # ---- static schedules / sem count maps ----
        # global group index: gg = (rep*2 + bt)*GQ + q
        n_bt = 2 * n_reps

        def gg_of(rep, bt, q):
            return (rep * 2 + bt) * GQ + q

        # chain-consumer sem value after the accumulate op for global group g.
        # GP emits _GP_OPS_BT ops per bt (adds + 2 pre-folds); DVE emits _N_DV
        # chain adds per bt (tail fold/merge ops do not inc dv_sem).
        cons = {}   # g -> list of (sem_name, value)
        for g in range(n_bt * GQ):
            blk_i, q = divmod(g, GQ)
            if _is_gp(q):
                cons[g] = [("gp", blk_i * _GP_OPS_BT + sum(
                    1 for qq in range(q + 1) if _is_gp(qq)))]
            else:
                cons[g] = [("dv", blk_i * _N_DV + sum(
                    1 for qq in range(q + 1) if _is_dv(qq)))]

        # one combined DMA per (parity, chunk): [even -> partitions 0:12,
        # odd -> 64:76].  Chunk 0's two DMAs run in parallel (sync + scalar).
        def _chunk_slices(acc, n_cb):
            lo, hi = acc * 768, (acc + n_cb) * 768
            return [(wrs[0:KK, lo:hi], wre[:, lo:hi]),
                    (wrs[64:64 + KK, lo:hi], wro[:, lo:hi])]

        @blk.sync
        def _(sync):
            c0 = _chunk_slices(0, CHUNK_PLAN[0])
            sync.dma_start(c0[0][0], c0[0][1]).then_inc(dma_sem, 16)
            acc = CHUNK_PLAN[0]
            for n_cb in CHUNK_PLAN[1:]:
                for dst, src in _chunk_slices(acc, n_cb):
                    sync.dma_start(dst, src).then_inc(dma_sem, 16)
                acc += n_cb
            for rep in range(n_reps):
                for bt in range(2):
                    sync.wait_ge(o_sem, rep * 2 + bt + 1)
                    sync.dma_start(y[bt * 128:(bt + 1) * 128, :], o_t[bt][:]
                                   ).then_inc(dma_sem, 16)

        @blk.tensor
        def _(tensor):
            # warm-up matmul (ungated, result overwritten by q0's start=True):
            # starts the PE p-state ramp before the first DMA lands
            tensor.matmul(dg[1][:, 0:512], wrs[0:KK, 0:128],
                          wrs[0:KK, 256:768], start=True, stop=True)
            for rep in range(n_reps):
                for bt in range(2):
                    blk_i = rep * 2 + bt
                    for q in range(GQ):
                        gg = gg_of(rep, bt, q)
                        if gg >= NDG:
                            # ACT must have consumed dg[gg % NDG]
                            tensor.wait_ge(act_sem, gg - NDG + 1)
                        for h in range(PPG):
                            p = q * PPG + h
                            if rep == 0 and bt == 0:
                                cb = p // 2
                                if cb < CHUNK_PLAN[0]:
                                    tensor.wait_ge(dma_sem, 16)
                                    tensor.wait_ge(dma0_sem, 16)
                                else:
                                    tensor.wait_ge(dma_sem, _cb_full_v[cb])
                            base = 64 * (p % 2)
                            col = (p // 2) * 768
                            tensor.matmul(
                                dg[gg % NDG][:, h * 512:(h + 1) * 512],
                                wrs[base:base + KK, col + bt * 128:col + bt * 128 + 128],
                                wrs[base:base + KK, col + 256:col + 768],
                                start=True, stop=True,
                            ).then_inc(mm_sem, 1)

        @blk.scalar
        def _(scalar):
            dst, src = _chunk_slices(0, CHUNK_PLAN[0])[1]
            scalar.dma_start(dst, src).then_inc(dma0_sem, 16)
            sems = {"gp": gp_sem, "dv": dv_sem}
            for rep in range(n_reps):
                for bt in range(2):
                    for q in range(GQ):
                        gg = gg_of(rep, bt, q)
                        if gg >= NRT:
                            # accumulation chain must have consumed rt[gg % NRT]
                            for sem_name, val in cons[gg - NRT]:
                                scalar.wait_ge(sems[sem_name], val)
                        if gg == 0:
                            # split the very first group so ScalarE starts
                            # after only 2 of its 4 matmuls have landed
                            scalar.wait_ge(mm_sem, 2)
                            _act_recip(nc, rt[0][:, 0:1024], dg[0][:, 0:1024],
                                       D_CONST)
                            scalar.wait_ge(mm_sem, 4)
                            _act_recip(nc, rt[0][:, 1024:2048], dg[0][:, 1024:2048],
                                       D_CONST).then_inc(act_sem, 1)
                            continue
                        scalar.wait_ge(mm_sem, PPG * (gg + 1))
                        _act_recip(nc, rt[gg % NRT][:], dg[gg % NDG][:], D_CONST
                                   ).then_inc(act_sem, 1)

        @blk.gpsimd
        def _(gpsimd):
            for rep in range(n_reps):
                for bt in range(2):
                    sg = s_gp[bt]
                    if rep > 0:
                        # previous rep's merge for this bt parity consumed sg
                        gpsimd.wait_ge(o_sem, 2 * (rep - 1) + bt + 1)
                    first = True
                    for q in range(GQ):
                        if not _is_gp(q):
                            continue
                        gg = gg_of(rep, bt, q)
                        gpsimd.wait_ge(act_sem, gg + 1)
                        if first:
                            gpsimd.tensor_copy(sg[:], rt[gg % NRT][:]
                                               ).then_inc(gp_sem, 1)
                        else:
                            gpsimd.tensor_tensor(sg[:], sg[:], rt[gg % NRT][:],
                                                 add).then_inc(gp_sem, 1)
                        first = False
                    # pre-fold own accumulator 2048 -> 256 (off the tail path)
                    gpsimd.tensor_tensor(sg[:, 0:1024], sg[:, 0:1024],
                                         sg[:, 1024:2048], add).then_inc(gp_sem, 1)
                    gpsimd.tensor_tensor(sg[:, 0:512], sg[:, 0:512],
                                         sg[:, 512:1024], add).then_inc(gp_sem, 1)
                    gpsimd.tensor_tensor(sg[:, 0:256], sg[:, 0:256],
                                         sg[:, 256:512], add).then_inc(gp_sem, 1)

        @blk.vector
        def _(vector):
            for rep in range(n_reps):
                for bt in range(2):
                    blk_i = rep * 2 + bt
                    first = True
                    for q in range(GQ):
                        if not _is_dv(q):
                            continue
                        gg = gg_of(rep, bt, q)
                        vector.wait_ge(act_sem, gg + 1)
                        if first:
                            vector.tensor_copy(s_dv[:], rt[gg % NRT][:]
                                               ).then_inc(dv_sem, 1)
                        else:
                            vector.tensor_tensor(s_dv[:], s_dv[:], rt[gg % NRT][:],
                                                 add).then_inc(dv_sem, 1)
                        first = False
                    # fold 2048 -> 256, add GP's 256-wide pre-folded result
                    vector.tensor_tensor(s_dv[:, 0:1024], s_dv[:, 0:1024],
                                         s_dv[:, 1024:2048], add)
                    vector.tensor_tensor(s_dv[:, 0:512], s_dv[:, 0:512],
                                         s_dv[:, 512:1024], add)
                    if rep > 0:
                        vector.wait_ge(dma_sem,
                                       16 * (IN_DMAS_SYNC + 2 * (rep - 1) + bt + 1))
                    vector.tensor_tensor(o_t[bt][:], s_dv[:, 0:256], s_dv[:, 256:512],
                                         add)
                    vector.wait_ge(gp_sem, (blk_i + 1) * _GP_OPS_BT)
                    vector.tensor_tensor(o_t[bt][:], o_t[bt][:],
                                         s_gp[bt][:, 0:256], add)
                    vector.tensor_scalar(o_t[bt][:], o_t[bt][:],
                                         -GAMMA, float(I_DIM), mult, add
                                         ).then_inc(o_sem, 1)

    return nc


_NC_CACHE = {}


def _get_nc(n_reps=1):
    if n_reps not in _NC_CACHE:
        _NC_CACHE[n_reps] = build_nc(n_reps)
    return _NC_CACHE[n_reps]


def kernel(input_matrix, phase_offset, n_reps=1):
    input_matrix = np.asarray(input_matrix, np.float32)
    phase_offset = np.asarray(phase_offset, np.float32)
    rr_pair = _host_rr(phase_offset)
    in_maps = []
    for c in range(N_CORES):
        we, wo = _host_wr(input_matrix[c * BL:(c + 1) * BL, :], rr_pair)
        in_maps.append({"WRE": we, "WRO": wo})
    nc = _get_nc(n_reps)
    res = run_bass_kernel_spmd(nc, in_maps, core_ids=list(range(N_CORES)))
    out = np.empty((B, O_DIM), np.float32)
    for c in range(N_CORES):
        out[c * BL:(c + 1) * BL, :] = res.results[c]["OUT"]
    return out


# revision 109
# speedup vs baseline: 1.0289x; 1.0050x over previous
"""PhotonicCore micro-ring transmission kernel for 8x TRN2 NeuronCores.

Math:
    phi = phi0 + X[b,j] + P[i,j]
    trans = (t^2 - 2at*cos(phi) + a^2) / (1 - 2at*cos(phi) + (at)^2)
          = 1 - gamma / d,   d = D - Bc*cos(phi),  D = 1+(at)^2, Bc = 2at,
            gamma = kappa*(1-a^2)
    out[b,i] = sum_j trans = 256 - gamma * sum_j 1/d[b,i,j]

Device strategy (data-parallel over batch, 256 rows/core):
    cos(phi) = cos(u)cos(p) - sin(u)sin(p)  with u = phi0 + X  ==>  for a
    fixed j, -Bc*cos(phi)[b,i] is a rank-2 outer product.  TensorE matmuls
    (K=12, fp16 hi/lo split => ~fp32-exact products) build -Bc*cos(phi)
    for 8 j-columns into a 4-bank PSUM group [128b x 2048].  ScalarE does
    the one mandatory per-element pass r = Reciprocal(psum + D) (measured
    2.7e-5 max rel err on our d range) into SBUF; the j-reduction runs as
    two independent accumulation chains on VectorE and GpSimdE (S += r),
    merged + folded + scaled by VectorE per 128-row block.
"""

import numpy as np

import concourse.bass as bass
import concourse.mybir as mybir
from concourse.bass_utils import run_bass_kernel_spmd

# ---- problem constants (hardcoded; kernel.py must be self-contained) ----
B, I_DIM, O_DIM = 2048, 256, 256
N_CORES = 8
BL = B // N_CORES  # 256 batch rows per core

KAPPA = 0.1
LOSS_A = 0.99
N_EFF = 3.48
LAMBDA = 1.55e-6
RADIUS = 5e-6

_t = np.sqrt(1.0 - KAPPA)
_a = LOSS_A
_x = _a * _t
BC = 2.0 * _a * _t
D_CONST = 1.0 + _x * _x
GAMMA = KAPPA * (1.0 - _a * _a)
PHI0 = float(np.float32((2.0 * np.pi * N_EFF * (2.0 * np.pi * RADIUS) / LAMBDA) % (2.0 * np.pi)))

N_PAIRS = 128   # j pairs per core (256 j / 2)
KK = 12         # contraction rows per matmul (6 per j)
PPG = 4         # pairs per PSUM group (4 pairs = 8 j = [128, 2048] = 4 banks)
GQ = N_PAIRS // PPG          # 32 groups per bt
NRT = 6         # r-tile rotation depth
NDG = 2         # PSUM d-group rotation depth


def _split16(v):
    hi = v.astype(np.float16)
    lo = (v - hi.astype(np.float64)).astype(np.float16)
    return hi, lo


def _pack2(per_pair, w):
    """Split per-pair [12, w] blocks into even/odd-pair compact tensors
    [12, 64, w] (DRAM stays compact; the DMA scatters even pairs to SBUF
    partition base 0 and odd pairs to base 64, where matmul can slice
    [12, w] blocks at legal partition bases)."""
    arr = per_pair.reshape(KK, N_PAIRS, w)
    return arr[:, 0::2, :], arr[:, 1::2, :]


def _host_wl(x_shard):
    """lhsT source: even/odd compact [12, 64*256] fp16 pair (see _pack2)."""
    u = PHI0 + x_shard.astype(np.float64)          # [256b, 256j]
    cu = -BC * np.cos(u)
    su = BC * np.sin(u)
    cu_hi, cu_lo = _split16(cu)
    su_hi, su_lo = _split16(su)
    rows = np.stack([cu_hi, cu_hi, cu_lo, su_hi, su_hi, su_lo], 0)  # [6,b,j]
    t_even = rows[:, :, 0::2].transpose(0, 2, 1)   # [6, 128p, 256b]
    t_odd = rows[:, :, 1::2].transpose(0, 2, 1)
    T = np.stack([t_even, t_odd], 0)               # [2, 6, 128p, 256b]
    # [h(2), s(6), p, b] -> [6h+s, p*256+b]
    return _pack2(T.reshape(KK, N_PAIRS * 256), 256)  # 2x [12, 64, 256]


def _host_wr(x_shard, rr_pair):
    """Combined per-parity operand tensors [12, 64*768] fp16: per col-block
    cb the layout is [wl(cb) 256 cols | rr(cb) 512 cols], so each input DMA
    chunk is ONE transfer per parity and matmul slices stay contiguous."""
    wl_e, wl_o = _host_wl(x_shard)
    rr_e, rr_o = rr_pair
    we = np.concatenate([wl_e, rr_e], axis=2).reshape(KK, 64 * 768)
    wo = np.concatenate([wl_o, rr_o], axis=2).reshape(KK, 64 * 768)
    return np.ascontiguousarray(we), np.ascontiguousarray(wo)


def _host_rr(phase_offset):
    """rhs source: even/odd compact [12, 64*512] fp16 pair, block-diagonal
    per j-pair (see _pack2)."""
    p = phase_offset.astype(np.float64)            # [256i, 256j]
    cp_hi, cp_lo = _split16(np.cos(p))
    sp_hi, sp_lo = _split16(np.sin(p))
    rows = np.stack([cp_hi, cp_lo, cp_hi, sp_hi, sp_lo, sp_hi], 0)  # [6, i, j]
    r_even = rows[:, :, 0::2].transpose(0, 2, 1)   # [6, 128p, 256i]
    r_odd = rows[:, :, 1::2].transpose(0, 2, 1)
    Z = np.zeros((2, 6, N_PAIRS, 2, 256), np.float16)      # [h, s, p, h', i]
    Z[0, :, :, 0, :] = r_even
    Z[1, :, :, 1, :] = r_odd
    return _pack2(Z.reshape(KK, N_PAIRS * 512), 512)  # 2x [12, 64, 512]


def _act_recip(nc, out, in_, bias):
    """out = 1/(in_ + bias) on ScalarE via direct InstActivation emission."""
    eng = nc.scalar
    inputs = [eng.lower_ap(in_)]
    for arg in [bias, 1.0, 0.0]:  # bias, scale, alpha
        inputs.append(mybir.ImmediateValue(dtype=mybir.dt.float32, value=float(arg)))
    return eng.add_instruction(
        mybir.InstActivation(
            name=nc.get_next_instruction_name(),
            func=mybir.ActivationFunctionType.Reciprocal,
            ins=inputs,
            outs=[eng.lower_ap(out)],
        )
    )


# which engine accumulates group q (within a bt): True -> GpSimd, else Vector.
# GP is ~2x slower per op than DVE: give it every 3rd group with slack, none
# near the bt end so it can pre-fold its accumulator off the critical tail.
def _is_gp(q):
    return q in (1, 4, 6, 9, 11, 14, 16, 19, 21, 24, 26, 28)


def _is_dv(q):
    return not _is_gp(q)


_N_GP = sum(1 for q in range(GQ) if _is_gp(q))    # GP groups per bt (12)
_N_DV = sum(1 for q in range(GQ) if _is_dv(q))    # DVE groups per bt (20)
_GP_OPS_BT = _N_GP + 3                            # adds + 3 pre-folds (to 256)


def build_nc(n_reps=1):
    """Build the per-core Bass graph. n_reps>1 repeats compute for timing."""
    f16, f32 = mybir.dt.float16, mybir.dt.float32
    add, mult = mybir.AluOpType.add, mybir.AluOpType.mult

    nc = bass.Bass("TRN2", target_bir_lowering=False, debug=False)
    wre = nc.dram_tensor("WRE", [KK, 64 * 768], f16, kind="ExternalInput").ap()
    wro = nc.dram_tensor("WRO", [KK, 64 * 768], f16, kind="ExternalInput").ap()
    y = nc.dram_tensor("OUT", [BL, 256], f32, kind="ExternalOutput").ap()

    # input DMA chunk plan over 64 col-blocks: one combined transfer per
    # parity per chunk.  Chunk 0's two DMAs go out in parallel on sync's and
    # scalar's queues (tracked on dma0_sem for the odd one); the remaining
    # chunks stream in-order on sync's queue with growing sizes.
    CHUNK_PLAN = [4, 4, 8, 16, 32]
    assert sum(CHUNK_PLAN) == 64
    IN_DMAS_SYNC = 1 + 2 * (len(CHUNK_PLAN) - 1)  # dma_sem incs from sync
    # cb -> dma_sem value once its (sync) chunk fully landed
    _cb_full_v = {}
    acc = CHUNK_PLAN[0]
    v = 16                      # sync's chunk-0 DMA
    for n_cb in CHUNK_PLAN[1:]:
        v += 32
        for cb in range(acc, acc + n_cb):
            _cb_full_v[cb] = v
        acc += n_cb

    # group 0 takes only even-parity pairs and group 1 only odd ones, so each
    # is gated on a single chunk-0 DMA (they complete ~1us apart); later
    # groups take consecutive pairs.  Pure permutation -- every pair's
    # columns are summed by the folds regardless of slot.
    PAIR_ORDER = [0, 2, 4, 6, 1, 3, 5, 7] + list(range(8, N_PAIRS))

    FD = PPG * 512   # 2048 free elems per group

    from contextlib import ExitStack
    with ExitStack() as st:
        ec = st.enter_context
        wrs = ec(nc.sbuf_tensor("wrs", [128, 64 * 768], f16))
        rt = [ec(nc.sbuf_tensor(f"rt{i}", [128, FD], f32)) for i in range(NRT)]
        s_dv = ec(nc.sbuf_tensor("s_dv", [128, FD], f32))
        s_gp = [ec(nc.sbuf_tensor(f"s_gp{i}", [128, FD], f32)) for i in range(2)]
        o_t = [ec(nc.sbuf_tensor(f"o{i}", [128, 256], f32)) for i in range(2)]
        dg = [ec(nc.psum_tensor(f"dg{i}", [128, FD], f32)) for i in range(NDG)]
        dma_sem = ec(nc.semaphore())
        dma0_sem = ec(nc.semaphore())  # chunk-0 odd DMA on scalar's queue
        mm_sem = ec(nc.semaphore())    # d-matmuls
        act_sem = ec(nc.semaphore())   # ACT recip groups
        dv_sem = ec(nc.semaphore())    # DVE chain adds
        gp_sem = ec(nc.semaphore())    # GP chain adds
        o_sem = ec(nc.semaphore())     # per-bt finished outputs
        blk = ec(nc.Block())

        # ---- static schedules / sem count maps ----
        # global group index: gg = (rep*2 + bt)*GQ + q
        n_bt = 2 * n_reps

        def gg_of(rep, bt, q):
            return (rep * 2 + bt) * GQ + q

        # chain-consumer sem value after the accumulate op for global group g.
        # GP emits _GP_OPS_BT ops per bt (adds + 2 pre-folds); DVE emits _N_DV
        # chain adds per bt (tail fold/merge ops do not inc dv_sem).
        cons = {}   # g -> list of (sem_name, value)
        for g in range(n_bt * GQ):
            blk_i, q = divmod(g, GQ)
            if _is_gp(q):
                cons[g] = [("gp", blk_i * _GP_OPS_BT + sum(
                    1 for qq in range(q + 1) if _is_gp(qq)))]
            else:
                cons[g] = [("dv", blk_i * _N_DV + sum(
                    1 for qq in range(q + 1) if _is_dv(qq)))]

        # one combined DMA per (parity, chunk): [even -> partitions 0:12,
        # odd -> 64:76].  Chunk 0's two DMAs run in parallel (sync + scalar).
        def _chunk_slices(acc, n_cb):
            lo, hi = acc * 768, (acc + n_cb) * 768
            return [(wrs[0:KK, lo:hi], wre[:, lo:hi]),
                    (wrs[64:64 + KK, lo:hi], wro[:, lo:hi])]

        @blk.sync
        def _(sync):
            c0 = _chunk_slices(0, CHUNK_PLAN[0])
            sync.dma_start(c0[0][0], c0[0][1]).then_inc(dma_sem, 16)
            acc = CHUNK_PLAN[0]
            for n_cb in CHUNK_PLAN[1:]:
                for dst, src in _chunk_slices(acc, n_cb):
                    sync.dma_start(dst, src).then_inc(dma_sem, 16)
                acc += n_cb
            for rep in range(n_reps):
                for bt in range(2):
                    sync.wait_ge(o_sem, rep * 2 + bt + 1)
                    sync.dma_start(y[bt * 128:(bt + 1) * 128, :], o_t[bt][:]
                                   ).then_inc(dma_sem, 16)

        @blk.tensor
        def _(tensor):
            # warm-up matmul (ungated, result overwritten by q0's start=True):
            # starts the PE p-state ramp before the first DMA lands
            tensor.matmul(dg[1][:, 0:512], wrs[0:KK, 0:128],
                          wrs[0:KK, 256:768], start=True, stop=True)
            for rep in range(n_reps):
                for bt in range(2):
                    blk_i = rep * 2 + bt
                    for q in range(GQ):
                        gg = gg_of(rep, bt, q)
                        if gg >= NDG:
                            # ACT must have consumed dg[gg % NDG]
                            tensor.wait_ge(act_sem, gg - NDG + 1)
                        for h in range(PPG):
                            p = PAIR_ORDER[q * PPG + h]
                            if rep == 0 and bt == 0:
                                cb = p // 2
                                if cb < CHUNK_PLAN[0]:
                                    if p % 2 == 0:
                                        tensor.wait_ge(dma_sem, 16)
                                    else:
                                        tensor.wait_ge(dma0_sem, 16)
                                else:
                                    tensor.wait_ge(dma_sem, _cb_full_v[cb])
                            base = 64 * (p % 2)
                            col = (p // 2) * 768
                            tensor.matmul(
                                dg[gg % NDG][:, h * 512:(h + 1) * 512],
                                wrs[base:base + KK, col + bt * 128:col + bt * 128 + 128],
                                wrs[base:base + KK, col + 256:col + 768],
                                start=True, stop=True,
                            ).then_inc(mm_sem, 1)

        @blk.scalar
        def _(scalar):
            dst, src = _chunk_slices(0, CHUNK_PLAN[0])[1]
            scalar.dma_start(dst, src).then_inc(dma0_sem, 16)
            sems = {"gp": gp_sem, "dv": dv_sem}
            for rep in range(n_reps):
                for bt in range(2):
                    for q in range(GQ):
                        gg = gg_of(rep, bt, q)
                        if gg >= NRT:
                            # accumulation chain must have consumed rt[gg % NRT]
                            for sem_name, val in cons[gg - NRT]:
                                scalar.wait_ge(sems[sem_name], val)
                        if gg == 0:
                            # split the very first group so ScalarE starts
                            # after only 2 of its 4 matmuls have landed
                            scalar.wait_ge(mm_sem, 2)
                            _act_recip(nc, rt[0][:, 0:1024], dg[0][:, 0:1024],
                                       D_CONST)
                            scalar.wait_ge(mm_sem, 4)
                            _act_recip(nc, rt[0][:, 1024:2048], dg[0][:, 1024:2048],
                                       D_CONST).then_inc(act_sem, 1)
                            continue
                        scalar.wait_ge(mm_sem, PPG * (gg + 1))
                        _act_recip(nc, rt[gg % NRT][:], dg[gg % NDG][:], D_CONST
                                   ).then_inc(act_sem, 1)

        @blk.gpsimd
        def _(gpsimd):
            for rep in range(n_reps):
                for bt in range(2):
                    sg = s_gp[bt]
                    if rep > 0:
                        # previous rep's merge for this bt parity consumed sg
                        gpsimd.wait_ge(o_sem, 2 * (rep - 1) + bt + 1)
                    first = True
                    for q in range(GQ):
                        if not _is_gp(q):
                            continue
                        gg = gg_of(rep, bt, q)
                        gpsimd.wait_ge(act_sem, gg + 1)
                        if first:
                            gpsimd.tensor_copy(sg[:], rt[gg % NRT][:]
                                               ).then_inc(gp_sem, 1)
                        else:
                            gpsimd.tensor_tensor(sg[:], sg[:], rt[gg % NRT][:],
                                                 add).then_inc(gp_sem, 1)
                        first = False
                    # pre-fold own accumulator 2048 -> 256 (off the tail path)
                    gpsimd.tensor_tensor(sg[:, 0:1024], sg[:, 0:1024],
                                         sg[:, 1024:2048], add).then_inc(gp_sem, 1)
                    gpsimd.tensor_tensor(sg[:, 0:512], sg[:, 0:512],
                                         sg[:, 512:1024], add).then_inc(gp_sem, 1)
                    gpsimd.tensor_tensor(sg[:, 0:256], sg[:, 0:256],
                                         sg[:, 256:512], add).then_inc(gp_sem, 1)

        @blk.vector
        def _(vector):
            for rep in range(n_reps):
                for bt in range(2):
                    blk_i = rep * 2 + bt
                    first = True
                    for q in range(GQ):
                        if not _is_dv(q):
                            continue
                        gg = gg_of(rep, bt, q)
                        vector.wait_ge(act_sem, gg + 1)
                        if first:
                            vector.tensor_copy(s_dv[:], rt[gg % NRT][:]
                                               ).then_inc(dv_sem, 1)
                        else:
                            vector.tensor_tensor(s_dv[:], s_dv[:], rt[gg % NRT][:],
                                                 add).then_inc(dv_sem, 1)
                        first = False
                    # fold 2048 -> 256, add GP's 256-wide pre-folded result
                    vector.tensor_tensor(s_dv[:, 0:1024], s_dv[:, 0:1024],
                                         s_dv[:, 1024:2048], add)
                    vector.tensor_tensor(s_dv[:, 0:512], s_dv[:, 0:512],
                                         s_dv[:, 512:1024], add)
                    if rep > 0:
                        vector.wait_ge(dma_sem,
                                       16 * (IN_DMAS_SYNC + 2 * (rep - 1) + bt + 1))
                    vector.tensor_tensor(o_t[bt][:], s_dv[:, 0:256], s_dv[:, 256:512],
                                         add)
                    vector.wait_ge(gp_sem, (blk_i + 1) * _GP_OPS_BT)
                    vector.tensor_tensor(o_t[bt][:], o_t[bt][:],
                                         s_gp[bt][:, 0:256], add)
                    vector.tensor_scalar(o_t[bt][:], o_t[bt][:],
                                         -GAMMA, float(I_DIM), mult, add
                                         ).then_inc(o_sem, 1)

    return nc


_NC_CACHE = {}


def _get_nc(n_reps=1):
    if n_reps not in _NC_CACHE:
        _NC_CACHE[n_reps] = build_nc(n_reps)
    return _NC_CACHE[n_reps]


def kernel(input_matrix, phase_offset, n_reps=1):
    input_matrix = np.asarray(input_matrix, np.float32)
    phase_offset = np.asarray(phase_offset, np.float32)
    rr_pair = _host_rr(phase_offset)
    in_maps = []
    for c in range(N_CORES):
        we, wo = _host_wr(input_matrix[c * BL:(c + 1) * BL, :], rr_pair)
        in_maps.append({"WRE": we, "WRO": wo})
    nc = _get_nc(n_reps)
    res = run_bass_kernel_spmd(nc, in_maps, core_ids=list(range(N_CORES)))
    out = np.empty((B, O_DIM), np.float32)
    for c in range(N_CORES):
        out[c * BL:(c + 1) * BL, :] = res.results[c]["OUT"]
    return out
